# revision 12
# baseline (speedup 1.0000x reference)
"""Trainium2 Bass kernel for nn_Decoder (6-layer decoder: masked self-attn,
encoder-decoder attn, bert-decoder attn, FFN, 3 LayerNorms per layer).

Sharding (8 NeuronCores): data-parallel over batch (4 pairs of cores) x
sequence-parallel over target positions within each pair (512 queries per
core). The only cross-core communication is one AllGather of the hidden
state per layer (pairs exchange 512-row halves so both sides can compute
full-sequence self-attention K/V). Layer 0 needs no collective (full x is
an input); cross-attention K/V come from encoder_out / bert_embedding which
every core holds in full.

Everything runs in "T-layout": activations [feature, seq] on [partition,
free]. GEMM Y = X @ W becomes matmul(out, lhsT=W, rhs=XT) -> YT with no
transposes anywhere. Attention scores are computed transposed ([key,
query]); softmax uses exp without max-subtraction (scores are O(10) so fp32
exp is safe; masked keys get -30000 bias -> exp == 0), key-padding via the
ACT bias operand, causal masking via a multiplicative 0/1 bf16 mask, and
the softmax denominator comes from a ones-augmented 65th V column so the
same matmul chain produces numerator and denominator. LayerNorm stats are
partition-axis sums via ones-vector matmuls; rsqrt is exp(-0.5*ln(v+eps))
so the whole kernel needs a single ACT table set (natural_log_exp).

Host-side folding: per-head out-projection wo/bo folds into the concat
matmul wc/bc (ho @ wc == ctx @ (wo @ wc)), and the 0.5*(ed+bd) averaging
folds into their wfold/bias, so each attention ends in one 1024x1024 GEMM
accumulated straight onto the residual. Weights are cast to bf16 on the
host (fp32 accumulation in PSUM); residual stream and LN stats stay fp32.
"""
import contextlib
import os
import sys

sys.path.insert(0, "/opt/trn_rl_repo")

import numpy as np
import ml_dtypes

import concourse.bass as bass  # noqa: F401
import concourse.mybir as mybir
import concourse.tile as tile
from concourse import bacc
from concourse.bass_utils import run_bass_kernel_spmd

AF = mybir.ActivationFunctionType
ALU = mybir.AluOpType
FP32 = mybir.dt.float32
BF16 = mybir.dt.bfloat16

L, H, DM, DFF, DB = 6, 16, 1024, 4096, 768
DK = DM // H  # 64
B, ST, SS = 4, 1024, 1024
EPS = 1e-5
NEG = -30000.0
P = 128
SQ = ST // 2          # queries per core
ND = DM // P          # 8
NF = DFF // P         # 32
NB = DB // P          # 6
NCORES = 8
BF_NP = ml_dtypes.bfloat16

bfq = lambda a: np.ascontiguousarray(np.asarray(a, dtype=BF_NP))
f32c = lambda a: np.ascontiguousarray(np.asarray(a, dtype=np.float32))


def _col_tiles(w):
    """[Din, E] -> [P, Din//P, E] partition-major tiling of the contraction
    axis; one per-partition-contiguous DMA per (layer, matrix)."""
    Din, E = w.shape
    return np.ascontiguousarray(w.reshape(Din // P, P, E).transpose(1, 0, 2))


def _part_vec(v):
    """[E] -> [P, E//P]: bias slice [P,1] per e-tile."""
    E = v.shape[0]
    return np.ascontiguousarray(v.reshape(E // P, P).T)


def fold_host(params):
    """Fold wo/bo into wc/bc and 0.5 into ed/bd; pack DMA-friendly layouts."""
    out = {}
    for name in ("sa", "ed", "bd"):
        mp = params[name]
        wq_l, wk_l, wv_l, wf_l = [], [], [], []
        bq_l, bk_l, bv_l, bc_l = [], [], [], []
        for l in range(L):
            wq = np.asarray(mp["wq"][l])  # [H, Din, DK]
            wk = np.asarray(mp["wk"][l])
            wv = np.asarray(mp["wv"][l])
            wo = np.asarray(mp["wo"][l])  # [H, DK, DK]
            wc = np.asarray(mp["wc"][l])  # [H*DK, DM]
            bo = np.asarray(mp["bo"][l])  # [H, DK]
            bc = np.asarray(mp["bc"][l])  # [DM]
            Din = wq.shape[1]
            wq_all = wq.transpose(1, 0, 2).reshape(Din, H * DK)
            wk_all = wk.transpose(1, 0, 2).reshape(wk.shape[1], H * DK)
            wv_all = wv.transpose(1, 0, 2).reshape(wv.shape[1], H * DK)
            wfold = np.concatenate(
                [wo[h] @ wc[h * DK : (h + 1) * DK, :] for h in range(H)], axis=0
            )
            bc_fold = bc + sum(bo[h] @ wc[h * DK : (h + 1) * DK, :] for h in range(H))
            s = 0.5 if name in ("ed", "bd") else 1.0
            wq_l.append(_col_tiles(wq_all))
            wk_l.append(_col_tiles(wk_all))
            wv_l.append(_col_tiles(wv_all))
            wf_l.append(_col_tiles(wfold * s))
            bq_l.append(_part_vec(np.asarray(mp["bq"][l]).reshape(-1)))
            bk_l.append(_part_vec(np.asarray(mp["bk"][l]).reshape(-1)))
            bv_l.append(_part_vec(np.asarray(mp["bv"][l]).reshape(-1)))
            bc_l.append(_part_vec(bc_fold * s))
        out[f"{name}_wq"] = bfq(np.stack(wq_l))  # [L, P, Din//P, H*DK]
        out[f"{name}_wk"] = bfq(np.stack(wk_l))
        out[f"{name}_wv"] = bfq(np.stack(wv_l))
        out[f"{name}_wf"] = bfq(np.stack(wf_l))  # [L, P, ND, DM]
        out[f"{name}_bq"] = f32c(np.stack(bq_l))  # [L, P, ND]
        out[f"{name}_bk"] = f32c(np.stack(bk_l))
        out[f"{name}_bv"] = f32c(np.stack(bv_l))
        out[f"{name}_bc"] = f32c(np.stack(bc_l))
    out["w1"] = bfq(np.stack([_col_tiles(np.asarray(params["w1"][l])) for l in range(L)]))
    out["w2"] = bfq(np.stack([_col_tiles(np.asarray(params["w2"][l])) for l in range(L)]))
    out["b1"] = f32c(np.stack([_part_vec(np.asarray(params["b1"][l])) for l in range(L)]))
    out["b2"] = f32c(np.stack([_part_vec(np.asarray(params["b2"][l])) for l in range(L)]))
    for i in (1, 2, 3):
        out[f"ln{i}_g"] = f32c(np.stack([_part_vec(np.asarray(params[f"ln{i}_g"][l])) for l in range(L)]))
        out[f"ln{i}_b"] = f32c(np.stack([_part_vec(np.asarray(params[f"ln{i}_b"][l])) for l in range(L)]))
    return out


def build_kernel(n_kt_self, n_kt_cross):
    nc = bacc.Bacc(None, target_bir_lowering=False, num_devices=NCORES)

    inp = {}

    def dparam(name, shape, dtype):
        inp[name] = nc.declare_dram_parameter(name, list(shape), dtype, isOutput=False)

    dparam("hT_own", [P, ND, SQ], FP32)
    dparam("xT_full", [P, ND, ST], BF16)
    dparam("encT", [P, ND, SS], BF16)
    dparam("bertT", [P, NB, SS], BF16)
    dparam("mask_self", [P, n_kt_self, SQ], BF16)
    dparam("kbias_self", [P, n_kt_self], FP32)
    dparam("kbias_cross", [P, n_kt_cross], FP32)
    for a in ("sa", "ed", "bd"):
        ndk = ND if a != "bd" else NB
        dparam(f"{a}_wq", [L, P, ND, DM], BF16)
        dparam(f"{a}_wk", [L, P, ndk, DM], BF16)
        dparam(f"{a}_wv", [L, P, ndk, DM], BF16)
        dparam(f"{a}_wf", [L, P, ND, DM], BF16)
        for bn in ("bq", "bk", "bv", "bc"):
            dparam(f"{a}_{bn}", [L, P, ND], FP32)
    dparam("w1", [L, P, ND, DFF], BF16)
    dparam("w2", [L, P, NF, DM], BF16)
    dparam("b1", [L, P, NF], FP32)
    dparam("b2", [L, P, ND], FP32)
    for i in (1, 2, 3):
        dparam(f"ln{i}_g", [L, P, ND], FP32)
        dparam(f"ln{i}_b", [L, P, ND], FP32)

    out_h = nc.declare_dram_parameter("out_hT", [P, ND, SQ], FP32, isOutput=True)

    with tile.TileContext(nc) as tc:
        build_body(nc, tc, inp, out_h, n_kt_self, n_kt_cross)
    nc.compile()
    return nc


def build_body(nc, tc, inp, out_h, n_kt_self, n_kt_cross):
    ctx = contextlib.ExitStack()
    with ctx:
        persist = ctx.enter_context(tc.tile_pool(name="persist", bufs=1))
        res = ctx.enter_context(tc.tile_pool(name="res", bufs=1))
        actbf = ctx.enter_context(tc.tile_pool(name="actbf", bufs=1))
        wstr = ctx.enter_context(tc.tile_pool(name="wstr", bufs=1))
        dram = ctx.enter_context(tc.tile_pool(name="dram", bufs=2, space="DRAM"))

        # ---------------- persistent loads ----------------------------
        mask_self = persist.tile([P, n_kt_self, SQ], BF16)
        nc.sync.dma_start(mask_self[:], inp["mask_self"][:, :, :])
        kb_self = persist.tile([P, n_kt_self], FP32)
        nc.sync.dma_start(kb_self[:], inp["kbias_self"][:, :])
        kb_cross = persist.tile([P, n_kt_cross], FP32)
        nc.sync.dma_start(kb_cross[:], inp["kbias_cross"][:, :])
        h_own = persist.tile([P, ND, SQ], FP32, tag="h_own", bufs=2)
        nc.sync.dma_start(h_own[:], inp["hT_own"][:, :, :])

        vecs = {}
        for name in ("sa_bq", "sa_bk", "sa_bv", "sa_bc", "ed_bq", "ed_bk",
                     "ed_bv", "ed_bc", "bd_bq", "bd_bk", "bd_bv", "bd_bc",
                     "b2", "ln1_g", "ln1_b", "ln2_g", "ln2_b", "ln3_g", "ln3_b"):
            t = persist.tile([P, L * ND], FP32, tag=f"vec_{name}")
            for l in range(L):
                nc.sync.dma_start(t[:, l * ND : (l + 1) * ND], inp[name][l])
            vecs[name] = t
        t = persist.tile([P, L * NF], FP32, tag="vec_b1")
        for l in range(L):
            nc.sync.dma_start(t[:, l * NF : (l + 1) * NF], inp["b1"][l])
        vecs["b1"] = t

        ones_col = persist.tile([P, 1], FP32, tag="ones_col")
        nc.any.memset(ones_col[:], 1.0)
        ones_row = persist.tile([1, P], FP32, tag="ones_row")
        nc.any.memset(ones_row[:], 1.0)

        # ---------------- helpers -------------------------------------
        def weight_tile(name, l):
            t = wstr.tile([P, ND, DM], BF16, tag="wstream", bufs=2)
            nc.sync.dma_start(t[:, : inp[name].shape[2], :], inp[name][l])
            return t

        def proj_T(psum_l, w_sb, rhs_fn, nd, bias_vec, voff, out_bf, ncols):
            """out_bf[:, et, c] = sum_d w[:,d,et*P:+P].T @ rhs(d)[:, c] + bias."""
            for et in range(ND):
                for ci in range((ncols + 511) // 512):
                    c0, c1 = ci * 512, min(ncols, (ci + 1) * 512)
                    ps = psum_l.tile([P, 512], FP32, tag="ps", bufs=3)
                    for dt in range(nd):
                        nc.tensor.matmul(
                            ps[:, : c1 - c0],
                            w_sb[:, dt, et * P : (et + 1) * P],
                            rhs_fn(dt)[:, c0:c1],
                            start=(dt == 0), stop=(dt == nd - 1),
                        )
                    nc.scalar.activation(
                        out_bf[:, et, c0:c1], ps[:, : c1 - c0], AF.Identity,
                        bias=bias_vec[:, voff + et : voff + et + 1],
                    )

        def attention(a, l, q_src_bf, kv_fn, nd_kv, n_kt, kbias, use_mask,
                      res_f32, out_r):
            """out_r (fp32 [P,ND,SQ]) = wfold.T @ ctxcat + bc (+ res_f32 or
            accumulate into existing out_r when res_f32 is None)."""
            voff = l * ND
            with (
                tc.tile_pool(name=f"at_{a}{l}", bufs=1) as loc,
                tc.tile_pool(name=f"atp_{a}{l}", bufs=1, space="PSUM") as psum_l,
            ):
                # -- projections --
                wq = weight_tile(f"{a}_wq", l)
                qT = loc.tile([P, ND, SQ], BF16, tag="qT")
                proj_T(psum_l, wq, lambda dt: q_src_bf[:, dt, :], ND,
                       vecs[f"{a}_bq"], voff, qT, SQ)
                wk = weight_tile(f"{a}_wk", l)
                kT = wstr.tile([P, ND, SS], BF16, tag="kT")
                proj_T(psum_l, wk, kv_fn, nd_kv, vecs[f"{a}_bk"], voff, kT, SS)
                # -- V natural + ones column --
                wv = weight_tile(f"{a}_wv", l)
                v65 = loc.tile([P, n_kt, H * 65], BF16, tag="v65")
                v65h = v65[:].rearrange("p s (h e) -> p s h e", e=65)
                nc.any.memset(v65h[:, :, :, 64:], 1.0)
                for st in range(n_kt):
                    for ci in range(2):
                        ps = psum_l.tile([P, 512], FP32, tag="ps", bufs=3)
                        for dt in range(nd_kv):
                            nc.tensor.matmul(
                                ps[:],
                                kv_fn(dt)[:, st * P : (st + 1) * P],
                                wv[:, dt, ci * 512 : (ci + 1) * 512],
                                start=(dt == 0), stop=(dt == nd_kv - 1),
                            )
                        nc.vector.tensor_copy(
                            v65h[:, st, ci * 8 : (ci + 1) * 8, :64],
                            ps[:].rearrange("p (h e) -> p h e", e=64),
                        )
                # -- per-head scores / softmax / AV / normalize --
                ctxcat = actbf.tile([P, ND, SQ], BF16, tag="ctxcat")
                for h in range(H):
                    et, prow = h // 2, (h % 2) * 64
                    ctx_ps = psum_l.tile([P, 512], FP32, tag="ctx", bufs=2)
                    for kt in range(n_kt):
                        s_ps = psum_l.tile([P, 512], FP32, tag="sps", bufs=2)
                        nc.tensor.matmul(
                            s_ps[:, :SQ],
                            kT[prow : prow + 64, et, kt * P : (kt + 1) * P],
                            qT[prow : prow + 64, et, :],
                            start=True, stop=True,
                        )
                        e_sb = loc.tile([P, SQ], BF16, tag="e_sb", bufs=3)
                        nc.scalar.activation(
                            e_sb[:], s_ps[:, :SQ], AF.Exp,
                            bias=kbias[:, kt : kt + 1], scale=0.125,
                        )
                        if use_mask:
                            nc.vector.tensor_tensor(
                                e_sb[:], e_sb[:], mask_self[:, kt, :], ALU.mult
                            )
                        nc.tensor.matmul(
                            ctx_ps[:65, :SQ],
                            v65h[:, kt, h, :],
                            e_sb[:],
                            start=(kt == 0), stop=(kt == n_kt - 1),
                        )
                    recip = loc.tile([1, SQ], FP32, tag="recip", bufs=2)
                    nc.vector.reciprocal(recip[:], ctx_ps[64:65, :SQ])
                    bc_ps = psum_l.tile([64, 512], FP32, tag="bc", bufs=1)
                    nc.tensor.matmul(bc_ps[:, :SQ],
                                     ones_row[:1, prow : prow + 64],
                                     recip[:], start=True, stop=True)
                    ctx_f = loc.tile([64, SQ], FP32, tag="ctx_f", bufs=1)
                    nc.scalar.activation(ctx_f[:], ctx_ps[:64, :SQ], AF.Copy)
                    nc.vector.tensor_tensor(ctx_f[:], ctx_f[:], bc_ps[:, :SQ],
                                            ALU.mult)
                    bv_s = vecs[f"{a}_bv"][prow : prow + 64,
                                           voff + et : voff + et + 1]
                    nc.vector.tensor_scalar(
                        ctxcat[prow : prow + 64, et, :], ctx_f[:], bv_s, None,
                        ALU.add,
                    )
                # -- wfold GEMM + bias + residual --
                wf = weight_tile(f"{a}_wf", l)
                for et in range(ND):
                    ps = psum_l.tile([P, 512], FP32, tag="ps", bufs=3)
                    for dt in range(ND):
                        nc.tensor.matmul(
                            ps[:, :SQ], wf[:, dt, et * P : (et + 1) * P],
                            ctxcat[:, dt, :],
                            start=(dt == 0), stop=(dt == ND - 1),
                        )
                    bcs = vecs[f"{a}_bc"][:, voff + et : voff + et + 1]
                    src = res_f32 if res_f32 is not None else out_r
                    nc.vector.tensor_tensor(out_r[:, et, :], ps[:, :SQ],
                                            src[:, et, :], ALU.add)
                    nc.vector.tensor_scalar(out_r[:, et, :], out_r[:, et, :],
                                            bcs, None, ALU.add)

        def layer_norm(r_t, g_vec, b_vec, voff, out_bf, out_f32):
            with (
                tc.tile_pool(name="ln_loc", bufs=1) as loc,
                tc.tile_pool(name="ln_psum", bufs=1, space="PSUM") as psl,
            ):
                sums = psl.tile([1, 2, 512], FP32, tag="sums", bufs=1)
                for dt in range(ND):
                    nc.tensor.matmul(sums[:, 0, :SQ], ones_col[:], r_t[:, dt, :],
                                     start=(dt == 0), stop=(dt == ND - 1))
                for dt in range(ND):
                    sq = loc.tile([P, SQ], FP32, tag="ln_sq", bufs=2)
                    nc.vector.tensor_tensor(sq[:], r_t[:, dt, :], r_t[:, dt, :],
                                            ALU.mult)
                    nc.tensor.matmul(sums[:, 1, :SQ], ones_col[:], sq[:],
                                     start=(dt == 0), stop=(dt == ND - 1))
                stats = loc.tile([1, 4, SQ], FP32, tag="ln_stats", bufs=1)
                mu, var, a_row, b_row = (stats[:, i, :] for i in range(4))
                nc.vector.tensor_scalar(mu, sums[:, 0, :SQ], 1.0 / DM, None,
                                        ALU.mult)
                nc.vector.tensor_scalar(var, sums[:, 1, :SQ], 1.0 / DM, None,
                                        ALU.mult)
                tmp1 = loc.tile([1, SQ], FP32, tag="ln_tmp", bufs=1)
                nc.vector.tensor_tensor(tmp1[:], mu, mu, ALU.mult)
                nc.vector.tensor_tensor(var, var, tmp1[:], ALU.subtract)
                nc.vector.tensor_scalar(var, var, EPS, None, ALU.add)
                nc.scalar.activation(a_row, var, AF.Ln)
                nc.scalar.activation(a_row, a_row, AF.Exp, scale=-0.5)
                nc.vector.tensor_tensor(b_row, mu, a_row, ALU.mult)
                nc.vector.tensor_scalar(b_row, b_row, -1.0, None, ALU.mult)
                ab = psl.tile([P, 2, SQ], FP32, tag="ab", bufs=1)
                nc.tensor.matmul(ab[:, 0, :], ones_row[:1, :], a_row,
                                 start=True, stop=True)
                nc.tensor.matmul(ab[:, 1, :], ones_row[:1, :], b_row,
                                 start=True, stop=True)
                for dt in range(ND):
                    tmp = loc.tile([P, SQ], FP32, tag="ln_norm", bufs=2)
                    nc.vector.tensor_tensor(tmp[:], r_t[:, dt, :], ab[:, 0, :],
                                            ALU.mult)
                    nc.vector.tensor_tensor(tmp[:], tmp[:], ab[:, 1, :], ALU.add)
                    gs = g_vec[:, voff + dt : voff + dt + 1]
                    bs = b_vec[:, voff + dt : voff + dt + 1]
                    nc.vector.tensor_scalar(out_bf[:, dt, :], tmp[:], gs, bs,
                                            ALU.mult, ALU.add)
                    if out_f32 is not None:
                        nc.vector.tensor_scalar(out_f32[:, dt, :], tmp[:], gs,
                                                bs, ALU.mult, ALU.add)

        # ------------------- the 6 layers ------------------------------
        h_full_dram = None
        cur_h_own = h_own
        for l in range(L):
            kvsrc = wstr.tile([P, ND, ST], BF16, tag="kvsrc")
            if l == 0:
                nc.sync.dma_start(kvsrc[:], inp["xT_full"][:, :, :])
            else:
                for dt in range(ND):
                    nc.sync.dma_start(kvsrc[:, dt, :SQ], h_full_dram[0, :, dt, :])
                    nc.sync.dma_start(kvsrc[:, dt, SQ:], h_full_dram[1, :, dt, :])

            h_own_bf = actbf.tile([P, ND, SQ], BF16, tag="h_bf", bufs=2)
            for dt in range(ND):
                nc.vector.tensor_copy(h_own_bf[:, dt, :], cur_h_own[:, dt, :])

            # ---- self attention + LN1 ----
            r1 = res.tile([P, ND, SQ], FP32, tag="res_f", bufs=2)
            attention("sa", l, h_own_bf, lambda dt: kvsrc[:, dt, :], ND,
                      n_kt_self, kb_self, True, cur_h_own, r1)
            h1_bf = actbf.tile([P, ND, SQ], BF16, tag="h_bf", bufs=2)
            h1_f = res.tile([P, ND, SQ], FP32, tag="res_f", bufs=2)
            layer_norm(r1, vecs["ln1_g"], vecs["ln1_b"], l * ND, h1_bf, h1_f)

            # ---- ed + bd cross attention + LN2 ----
            kvsrc = wstr.tile([P, ND, ST], BF16, tag="kvsrc")
            nc.sync.dma_start(kvsrc[:], inp["encT"][:, :, :])
            r2 = res.tile([P, ND, SQ], FP32, tag="res_f", bufs=2)
            attention("ed", l, h1_bf, lambda dt: kvsrc[:, dt, :], ND,
                      n_kt_cross, kb_cross, False, h1_f, r2)
            kvsrc = wstr.tile([P, ND, ST], BF16, tag="kvsrc")
            nc.sync.dma_start(kvsrc[:, :NB, :], inp["bertT"][:, :, :])
            attention("bd", l, h1_bf, lambda dt: kvsrc[:, dt, :], NB,
                      n_kt_cross, kb_cross, False, None, r2)
            h2_bf = actbf.tile([P, ND, SQ], BF16, tag="h_bf", bufs=2)
            h2_f = res.tile([P, ND, SQ], FP32, tag="res_f", bufs=2)
            layer_norm(r2, vecs["ln2_g"], vecs["ln2_b"], l * ND, h2_bf, h2_f)

            # ---- FFN + LN3 ----
            with (
                tc.tile_pool(name=f"ffn{l}", bufs=1) as floc,
                tc.tile_pool(name=f"ffnp{l}", bufs=1, space="PSUM") as psf,
            ):
                f1 = floc.tile([P, NF, SQ], BF16, tag="f1")
                for wc_i in range(4):
                    w1c = wstr.tile([P, ND, 1024], BF16, tag="wstream", bufs=2)
                    nc.sync.dma_start(
                        w1c[:], inp["w1"][l, :, :, wc_i * 1024 : (wc_i + 1) * 1024]
                    )
                    for fi in range(8):
                        ft = wc_i * 8 + fi
                        ps = psf.tile([P, 512], FP32, tag="ps", bufs=2)
                        for dt in range(ND):
                            nc.tensor.matmul(
                                ps[:, :SQ], w1c[:, dt, fi * P : (fi + 1) * P],
                                h2_bf[:, dt, :],
                                start=(dt == 0), stop=(dt == ND - 1),
                            )
                        nc.scalar.activation(
                            f1[:, ft, :], ps[:, :SQ], AF.Relu,
                            bias=vecs["b1"][:, l * NF + ft : l * NF + ft + 1],
                        )
                r3 = res.tile([P, ND, SQ], FP32, tag="res_f", bufs=2)
                for eg in range(2):
                    w2_ps = [psf.tile([P, 512], FP32, tag="w2ps", bufs=4,
                                      name=f"w2ps_{eg}_{i}")
                             for i in range(4)]
                    for dc in range(4):
                        w2c = wstr.tile([P, ND, 1024], BF16, tag="wstream", bufs=2)
                        nc.sync.dma_start(
                            w2c[:], inp["w2"][l, :, dc * 8 : (dc + 1) * 8, :]
                        )
                        for di in range(8):
                            ft = dc * 8 + di
                            for ei in range(4):
                                et = eg * 4 + ei
                                nc.tensor.matmul(
                                    w2_ps[ei][:, :SQ],
                                    w2c[:, di, et * P : (et + 1) * P],
                                    f1[:, ft, :],
                                    start=(ft == 0), stop=(ft == NF - 1),
                                )
                    for ei in range(4):
                        et = eg * 4 + ei
                        nc.vector.tensor_tensor(r3[:, et, :], w2_ps[ei][:, :SQ],
                                                h2_f[:, et, :], ALU.add)
                        nc.vector.tensor_scalar(
                            r3[:, et, :], r3[:, et, :],
                            vecs["b2"][:, l * ND + et : l * ND + et + 1], None,
                            ALU.add,
                        )
            h3_bf = actbf.tile([P, ND, SQ], BF16, tag="h_bf", bufs=2)
            nxt_h_own = persist.tile([P, ND, SQ], FP32, tag="h_own", bufs=2)
            layer_norm(r3, vecs["ln3_g"], vecs["ln3_b"], l * ND, h3_bf, nxt_h_own)
            cur_h_own = nxt_h_own

            # ---- AllGather h3 within pairs ----
            if l < L - 1:
                cc_in = dram.tile([P, ND, SQ], BF16)
                cc_out = dram.tile([2, P, ND, SQ], BF16)
                nc.sync.dma_start(cc_in[:], h3_bf[:])
                nc.gpsimd.collective_compute(
                    "AllGather", ALU.bypass,
                    replica_groups=[[0, 1], [2, 3], [4, 5], [6, 7]],
                    ins=[cc_in[:].opt()],
                    outs=[cc_out[:].opt()],
                )
                h_full_dram = cc_out

        nc.sync.dma_start(out_h[:, :, :], cur_h_own[:])




def part3_np(arr2d, nt):
    """[D, S] -> [P, nt, S] partition-major."""
    D, S = arr2d.shape
    assert D == nt * P
    return np.ascontiguousarray(arr2d.reshape(nt, P, S).transpose(1, 0, 2))

_BUILD_CACHE = {}


def kernel(x, encoder_out, bert_embedding, src_padding_mask, tgt_padding_mask,
           params):
    x = np.asarray(x)
    encoder_out = np.asarray(encoder_out)
    bert_embedding = np.asarray(bert_embedding)
    src_padding_mask = np.asarray(src_padding_mask)
    tgt_padding_mask = np.asarray(tgt_padding_mask)

    folded = fold_host(params)

    causal = np.triu(np.ones((ST, ST), bool), 1)

    def n_live_tiles(allow):
        idx = np.nonzero(allow)[0]
        last = int(idx.max()) if idx.size else 0
        return max(1, int(np.ceil((last + 1) / P)))

    n_kt_self = max(n_live_tiles(~tgt_padding_mask[b, 0]) for b in range(B))
    n_kt_cross = max(n_live_tiles(~src_padding_mask[b, 0]) for b in range(B))

    key = (n_kt_self, n_kt_cross)
    if key not in _BUILD_CACHE:
        _BUILD_CACHE[key] = build_kernel(n_kt_self, n_kt_cross)
    nc = _BUILD_CACHE[key]

    part3 = part3_np

    in_maps = []
    for c in range(NCORES):
        b, half = c // 2, c % 2
        q0 = half * SQ
        xT = x[b].T.astype(np.float32)
        tgt_pad = tgt_padding_mask[b, 0]
        src_pad = src_padding_mask[b, 0]
        self_masked = causal | tgt_pad[None, :]
        mm = (~self_masked[q0 : q0 + SQ, : n_kt_self * P]).T.astype(np.float32)
        kb_self = np.where(tgt_pad[: n_kt_self * P], NEG, 0.0).astype(np.float32)
        kb_cross = np.where(src_pad[: n_kt_cross * P], NEG, 0.0).astype(np.float32)
        m = {
            "hT_own": part3(xT[:, q0 : q0 + SQ], ND),
            "xT_full": bfq(part3(xT, ND)),
            "encT": bfq(part3(encoder_out[b].T.astype(np.float32), ND)),
            "bertT": bfq(part3(bert_embedding[b].T.astype(np.float32), NB)),
            "mask_self": bfq(part3(mm, n_kt_self)),
            "kbias_self": f32c(kb_self.reshape(n_kt_self, P).T),
            "kbias_cross": f32c(kb_cross.reshape(n_kt_cross, P).T),
        }
        m.update(folded)
        in_maps.append(m)

    res = run_bass_kernel_spmd(nc, in_maps, core_ids=list(range(NCORES)),
                               trace=bool(os.environ.get("KERNEL_TRACE")))
    out = np.zeros((B, ST, DM), np.float32)
    for c in range(NCORES):
        b, half = c // 2, c % 2
        q0 = half * SQ
        hT = res.results[c]["out_hT"]
        out[b, q0 : q0 + SQ, :] = hT.transpose(1, 0, 2).reshape(DM, SQ).T
    kernel._last_results = res
    return out
